# revision 22
# baseline (speedup 1.0000x reference)
"""HadLinear TRN2 kernel: out = fwht_1024blocks(x)/sqrt(1024) @ W.T

Math: the blockwise FWHT is multiplication by a symmetric matrix
(blockdiag of H_1024 = H_2^{x10}), so it folds into the weight:
    y = fwht(x)/32 @ W^T = x @ (fwht(W)/32)^T
The weight transform is done once on host (numpy); the device kernel is
a pure bf16 matmul, data-parallel over rows of x (2048 rows/core):
    y_core[2048, 4096] = x_core[2048, 4096] @ Wh[4096, 4096]^T

Device layout (all DMA contiguous, no transposes on device):
  xt  [4096(k), 2048(m)] bf16   - x_core^T, host-transposed
  wt  [4096(k), 4096(n)] bf16   - (fwht(W)/32)^T, host-prepared
  y   [2048(m), 4096(n)] f32

Loop: n-strips of 512 (one PSUM bank per out tile). W streamed once
(strip by strip), x fully SBUF-resident (16MB), y stores on the gpsimd
DMA queue so W prefetch is never blocked.  Strip 0 is structured so the
PE is busy from ~+6us and never waits on the initial x load:
  phase A  (m 0..7):  k-outer over 8 PSUM banks, paced by the
                      (xA slab, W0 tile) DMA pair stream (1.6us DMA vs
                      1.73us of matmul per k-slab)
  phase B1 (m 8..13): k-outer over 6 banks -> consumes the xB slab
                      stream at 1.3us/slab, slower than its arrival
  phase B2 (m 14,15) and strips 1..7: m-outer / k-inner, 32 chained
                      matmuls per bank, evict right after each chain
Measured ~904us/core (PE ~97.5% busy, matmuls at the 216ns N=512
streaming floor; bf16 peak would be 884us + ~20us fixed overhead).

Self-contained: hardcodes shapes B=4, S=4096, D_in=D_out=4096, 8 cores.
"""

import math
import numpy as np
import ml_dtypes

import concourse.bacc as bacc
import concourse.mybir as mybir
import concourse.tile as tile
from concourse.bass_utils import run_bass_kernel_spmd

BF16 = ml_dtypes.bfloat16

P = 128
N_CORES = 8
B_FULL, S_FULL, D = 4, 4096, 4096
M_FULL = B_FULL * S_FULL          # 16384 rows total
M_CORE = M_FULL // N_CORES        # 2048 rows per core
HAD = 1024                        # hadamard block
NSTRIP = 512                      # out-feature strip width (PSUM bank)

KT = D // P                       # 32 k-tiles
MT = M_CORE // P                  # 16 m-tiles
NS = D // NSTRIP                  # 8 n-strips


def build_nc():
    f32, bf16 = mybir.dt.float32, mybir.dt.bfloat16
    nc = bacc.Bacc(None, target_bir_lowering=False, debug=False)

    xt = nc.declare_dram_parameter("xt", [D, M_CORE], bf16, isOutput=False)
    wt = nc.declare_dram_parameter("wt", [D, D], bf16, isOutput=False)
    y = nc.declare_dram_parameter("y", [M_CORE, D], f32, isOutput=True)

    MH = M_CORE // 2              # 1024: x column-half width
    with tile.TileContext(nc) as tc:
        with (
            tc.tile_pool(name="xp", bufs=2 * KT) as xp,      # 64 x 2KB/part
            tc.tile_pool(name="wp", bufs=2 * KT) as wp,      # 64 x 1KB/part
            tc.tile_pool(name="op", bufs=6) as op,           # 6 x 2KB/part
            tc.tile_pool(name="x0p", bufs=3) as x0p,         # 3 x 1KB/part
            tc.tile_pool(name="ps", bufs=8, space="PSUM") as psp,
        ):
            # Startup DMA order (single queue): (xA slab, W0 tile) pairs in
            # k order, then xB slabs.  Strip-0 phase A consumes the pairs
            # at 1.31us/kt DMA vs 1.73us/kt of matmul -> PE-bound from the
            # first k-slab; phase B then consumes xB at 0.73us/kt.
            # [128, 1024] x-slabs give 2KB DMA lines (full HBM rate).
            xa, xb, w0tiles = [], [], []
            # HAM warmup: the PE idles ~6us at kernel entry waiting on the
            # first DMA's completion semaphore, and its clock gate only
            # releases (1.2 -> 2.4 GHz) after ~3.4us of sustained activity.
            # Burn that dead window with matmuls on a zeroed tile into a
            # scratch PSUM bank (the first real chain's start=True clears
            # it) so the real matmuls start at full clock.
            # 6 matmuls x 427ns (cold) span the idle window up to the
            # ~+10.4us first-DMA gate without pushing the real start later;
            # the PE stays busy seamlessly into the real matmuls, so the
            # clock gate still releases ~3.4us after the chain begins.
            warm = x0p.tile([P, NSTRIP], bf16, tag="x0", name="warm")
            nc.vector.memset(warm[:], 0)
            psw = psp.tile([P, NSTRIP], f32, tag="ps", name="ps_warm")
            for i in range(6):
                nc.tensor.matmul(psw[:], lhsT=warm[:, 0:P], rhs=warm[:],
                                 start=(i == 0), stop=(i == 5))
            # bootstrap: tiny first tiles so the first matmul issues ~4us
            # earlier than a full slab load would allow
            # three idle queues in parallel: their completion semaphores
            # (issue + transfer + ~4.3us latency) land ~together instead of
            # serializing, and the sync queue starts the pair stream at once
            x00 = []
            for h, q in ((0, nc.gpsimd), (1, nc.scalar)):
                t = x0p.tile([P, NSTRIP], bf16, tag="x0", name=f"x00_{h}")
                q.dma_start(
                    out=t[:], in_=xt[0:P, h * NSTRIP:(h + 1) * NSTRIP])
                x00.append(t)
            w = wp.tile([P, NSTRIP], bf16, tag="w", name="w_0_0")
            nc.sync.dma_start(out=w[:], in_=wt[0:P, 0:NSTRIP])
            w0tiles.append(w)
            # xa[0] is only read from strip 1 on (phase A kt0 uses the
            # bootstrap tiles), so its full-slab load goes after the pair
            # stream -- every early k-tile gains ~0.7us of DMA lead.
            xa.append(None)
            for kt_i in range(1, KT):
                t = xp.tile([P, MH], bf16, tag="x", name=f"xa_{kt_i}")
                nc.sync.dma_start(out=t[:],
                                  in_=xt[kt_i * P:(kt_i + 1) * P, 0:MH])
                xa.append(t)
                w = wp.tile([P, NSTRIP], bf16, tag="w",
                            name=f"w_0_{kt_i}")
                nc.sync.dma_start(
                    out=w[:], in_=wt[kt_i * P:(kt_i + 1) * P, 0:NSTRIP])
                w0tiles.append(w)
            t = xp.tile([P, MH], bf16, tag="x", name="xa_0")
            nc.sync.dma_start(out=t[:], in_=xt[0:P, 0:MH])
            xa[0] = t
            for kt_i in range(KT):
                t = xp.tile([P, MH], bf16, tag="x", name=f"xb_{kt_i}")
                nc.sync.dma_start(out=t[:],
                                  in_=xt[kt_i * P:(kt_i + 1) * P, MH:M_CORE])
                xb.append(t)

            def lhs(kt_i, m):
                half, sub = divmod(m, MT // 2)
                src = xa[kt_i] if half == 0 else xb[kt_i]
                return src[:, sub * P:(sub + 1) * P]

            def evict(ps_tile, m, ns):
                # alternate scalar/vector so eviction copies of adjacent
                # banks run in parallel (different PSUM banks: legal)
                cout = op.tile([P, NSTRIP], f32, tag="o", name=f"o_{ns}_{m}")
                if m % 2 == 0:
                    nc.scalar.copy(out=cout[:], in_=ps_tile[:])
                else:
                    nc.vector.tensor_copy(out=cout[:], in_=ps_tile[:])
                # scalar queue: keeps y stores off the x/W load queue, and
                # its end-of-kernel DMA-ring drain is ~3us cheaper than
                # gpsimd's
                nc.scalar.dma_start(
                    out=y[m * P:(m + 1) * P, ns * NSTRIP:(ns + 1) * NSTRIP],
                    in_=cout[:])

            for ns in range(NS):
                n0 = ns * NSTRIP
                if ns == 0:
                    wtiles = w0tiles
                else:
                    wtiles = []
                    for kt_i in range(KT):
                        t = wp.tile([P, NSTRIP], bf16, tag="w",
                                    name=f"w_{ns}_{kt_i}")
                        nc.sync.dma_start(
                            out=t[:], in_=wt[kt_i * P:(kt_i + 1) * P,
                                             n0:n0 + NSTRIP])
                        wtiles.append(t)

                if ns == 0:
                    # Phase A: k-outer over 8-bank m-group 0..7 -> compute
                    # starts on the first (x, w) k-slab instead of waiting
                    # for all of x.  Evictions fire per-bank right after
                    # that bank's last matmul.
                    pss = [psp.tile([P, NSTRIP], f32, tag="ps",
                                    name=f"ps0_{g}") for g in range(8)]
                    for kt_i in range(KT):
                        last = kt_i == KT - 1
                        for g in range(8):
                            if kt_i == 0:
                                lt = x00[g // 4][:, (g % 4) * P:
                                                 (g % 4 + 1) * P]
                            else:
                                lt = lhs(kt_i, g)
                            nc.tensor.matmul(
                                pss[g][:],
                                lhsT=lt,
                                rhs=wtiles[kt_i][:],
                                start=(kt_i == 0), stop=last)
                            if last:
                                evict(pss[g], g, ns)
                    # Phase B part 1: k-outer over 6 banks (m 8..13) so xB
                    # is consumed at 1.3us/slab -- slower than its DMA
                    # arrival rate, so the PE never waits on the xB tail.
                    pssb = [psp.tile([P, NSTRIP], f32, tag="ps",
                                     name=f"psb_{g}") for g in range(6)]
                    for kt_i in range(KT):
                        last = kt_i == KT - 1
                        for g in range(6):
                            nc.tensor.matmul(
                                pssb[g][:],
                                lhsT=lhs(kt_i, 8 + g),
                                rhs=wtiles[kt_i][:],
                                start=(kt_i == 0), stop=last)
                            if last:
                                evict(pssb[g], 8 + g, ns)
                    # Phase B part 2: the last two m-tiles, m-outer
                    m_range = range(MT - 2, MT)
                else:
                    m_range = range(MT)
                # m-outer, k-inner: 32 chained matmuls per PSUM bank
                for m in m_range:
                    ps = psp.tile([P, NSTRIP], f32, tag="ps",
                                  name=f"ps_{ns}_{m}")
                    for kt_i in range(KT):
                        nc.tensor.matmul(
                            ps[:],
                            lhsT=lhs(kt_i, m),
                            rhs=wtiles[kt_i][:],
                            start=(kt_i == 0), stop=(kt_i == KT - 1))
                    evict(ps, m, ns)
    nc.compile()
    return nc


_CACHE = {}


def _get_nc():
    if "nc" not in _CACHE:
        _CACHE["nc"] = build_nc()
    return _CACHE["nc"]


def _fwht_rows(a):
    """In-place FWHT along last axis (matches reference ordering).

    Mutates `a` -- callers must pass an owned copy."""
    orig = a.shape
    n = orig[-1]
    a = a.reshape(-1, n)
    h = 1
    while h < n:
        v = a.reshape(-1, 2, h)
        s = v[:, 0, :] + v[:, 1, :]
        d = v[:, 0, :] - v[:, 1, :]
        v[:, 0, :] = s
        v[:, 1, :] = d
        h *= 2
    return a.reshape(orig)


def _prep_inputs(x, weight):
    """Host prep: fold FWHT into W, transpose + cast to bf16."""
    x2d = np.asarray(x, dtype=np.float32).reshape(M_FULL, D)
    # explicit copy: _fwht_rows works in place and must not touch the
    # caller's weight array
    w = np.array(weight, dtype=np.float32, copy=True)

    wh = _fwht_rows(w.reshape(D, D // HAD, HAD)).reshape(D, D)
    wh *= 1.0 / math.sqrt(HAD)
    wt_bf = np.ascontiguousarray(wh.T).astype(BF16)

    xbf = x2d.astype(BF16)
    xts = [np.ascontiguousarray(xbf[c * M_CORE:(c + 1) * M_CORE, :].T)
           for c in range(N_CORES)]
    return xts, wt_bf


def run(x, weight, trace=False):
    assert x.shape == (B_FULL, S_FULL, D) and weight.shape == (D, D)
    nc = _get_nc()
    xts, wt_bf = _prep_inputs(x, weight)
    in_maps = [{"xt": xts[c], "wt": wt_bf} for c in range(N_CORES)]
    res = run_bass_kernel_spmd(nc, in_maps, core_ids=list(range(N_CORES)),
                               trace=trace)
    yv = np.concatenate([r["y"] for r in res.results], axis=0)
    return yv.reshape(B_FULL, S_FULL, D), res


def kernel(x, weight):
    return run(x, weight)[0]
